# revision 18
# baseline (speedup 1.0000x reference)
"""Trainium2 Bass kernel for nn_BaselineDistiller: grouped-expert MLP + MSE loss.

reference:
    h    = einsum('bne,neh->bnh', features, W1) + b1
    g    = gelu(h)                      # exact (erf) gelu
    pred = einsum('bnh,nhe->bne', g, W2) + b2
    out  = mean((pred - target)^2)

Strategy (8 NeuronCores, data-parallel over batch; ~151.5us on HW):
  ScalarE(gelu)-bound: 131072 gelu elems/partition/core at 1 elem/cycle
  /1.2GHz = 109us floor; PSUM (8 banks, 2KiB each) limits ACT ops to
  FD-1024, so the gelu stream floor is 128 ops x ~1005ns = 128.6us plus
  ~33 ops that run +110ns when a feat/targ DMA lands mid-op. Everything
  else is arranged to hide under that stream:
  * All inputs fp8 e4m3 (feat*8, targ*16 with b2 folded, W1*8, W2*16).
    mm1 (h.T = W1c.T @ feat.T per 128-row chunk) writes PSUM; ACT applies
    gelu with per-chunk bias b1 and scale 1/64 in 4 FD-1024 ops/expert,
    writing fp8 hact as two tiles (t01/t23 pairs; split so the tail
    flushes do not pick up a whole-tile dep on the last ACT op) laid out
    [128, 2t, 2k, 512] so mm2 runs as ONE fp8 DoubleRow matmul per tile
    (K=256). pred tiles accumulate [W2-DR, -I @ (16*(targ-b2))]; one DVE
    bn_stats per tile yields per-256-group {n, mean, M2}; the host
    reconstructs sum(diff^2) = sum M2 + n*mean^2 (dividing out 16^2).
  * PSUM: ph pool (2 bufs x 2 banks) mm1->ACT ping-pong; pp pool
    (2 bufs x 2 banks) pred pairs; every cross-engine chain has >=1 full
    ACT window of slack (fresher deps stall the in-order PE queue and
    open ACT gaps - measured; also the tile scheduler coalesces each
    flush's ACT dep to "all ACT ops emitted so far").
  * Ramp: the two HWDGE rings split the critical transfers (sync:
    feat0 as two tiles + feat1-3 + targ0; scalar/Act: head1 = b1+W1[e0]
    first, then gelu table load + dummy op, then head2 = negI+W2[e0] and
    the w1/w2 group-0 blocks). The ramp is chip-HBM-bound (8 cores ramp
    together, ~2us completion latency per transfer), so transfers are
    deadline-ordered in the ring FIFOs rather than spread wide; no SWDGE
    (a gpsimd dma inserts a multi-us DRAIN that delays the PE warm-up).
    10 junk matmuls hold the PE p-state until feat0 lands; steady-state
    io (feat+targ combined per expert - fewer transfers, same byte-
    proportional ACT landing penalty), weight-group and stats DMAs all
    ride the sync ring, with io prefetch depth 2 during experts 0-1
    (depth 3 after) - the extra
    in-flight ramp transfers otherwise slow e0/e1's mm1s and open
    ~0.3-0.5us of ACT gaps. Measured stream-internal gaps: ZERO >50ns.
  * Tail: e31's t01 pair is flushed during its own last two windows and
    stats ship in 96/30/2 slot splits, so after the final gelu only
    [t2,t3 mm2 + 2 bn_stats + a 6KB stats DMA] remain before the fixed
    (~8us) NEFF postamble.
  Typical HW time ~151.5-152us at nominal clocks; the chip DVFS-throttles some
  runs ~20% (the schedule stays gap-free either way: stream-internal ACT
  gaps measured ~0.1-0.7us total).
"""

import contextlib
import ctypes
import json
import sys
import types

import ml_dtypes
import numpy as np

import concourse.bass as bass
import concourse.mybir as mybir
import concourse.tile as tile
from concourse import bass_utils
from concourse.bass import ts
from concourse.bass_utils import run_bass_kernel_spmd

B, NE, E, H = 16384, 32, 128, 256
C = 8              # cores
BS = B // C        # batch rows per core
BT = 512           # batch columns per matmul tile
NT = BS // BT      # 4
FP8 = mybir.dt.float8e4
F32 = mybir.dt.float32
DR = mybir.MatmulPerfMode.DoubleRow

S_X = 8.0          # feature scale into fp8
S_W1 = 8.0
S_W2 = 16.0        # also the target scale (so pred/targ match in PSUM)
ACT_SCALE = 1.0 / (S_X * S_W1)
TTR_SCALE = 1.0 / (S_W2 * S_W2)

# ---------------------------------------------------------------------------
# Environment shims (idempotent):
#  1. antenv.axon_hooks — the image's antenv lacks it; provide the NTFF
#     profile hook via ctypes so trace=True works when a caller requests it.
#  2. upload_artifacts — no bucket access in this container; keep local.
#  3. This walrus build rejects instructions with >1 sync-wait; split the
#     extra waits onto NoOps at BIR-serialization time.
# ---------------------------------------------------------------------------
_AXON_SO = "/opt/axon/libaxon_pjrt.so"


def _make_ntff_hook(so_path):
    try:
        lib = ctypes.CDLL(so_path)
    except OSError:
        return None
    if not hasattr(lib, "axon_start_nrt_profile"):
        return None
    lib.axon_start_nrt_profile.argtypes = [ctypes.POINTER(ctypes.c_int64), ctypes.c_size_t]
    lib.axon_start_nrt_profile.restype = ctypes.c_int64
    lib.axon_stop_nrt_profile.argtypes = [ctypes.c_char_p]
    lib.axon_stop_nrt_profile.restype = ctypes.c_int64

    @contextlib.contextmanager
    def _hook(output_dir, device_ids):
        import jax

        jax.devices()
        if device_ids:
            ids = (ctypes.c_int64 * len(device_ids))(*device_ids)
            rc = lib.axon_start_nrt_profile(ids, len(device_ids))
        else:
            rc = lib.axon_start_nrt_profile(None, 0)
        if rc != 0:
            raise RuntimeError(f"axon_start_nrt_profile rc={rc}")
        try:
            yield
        finally:
            n = lib.axon_stop_nrt_profile(str(output_dir).encode())
            print(f"profile: {n} file(s) written to {output_dir}", file=sys.stderr)

    return _hook


if "antenv.axon_hooks" not in sys.modules:
    _mod = types.ModuleType("antenv.axon_hooks")
    _the_hook = _make_ntff_hook(_AXON_SO)
    _mod.get_axon_ntff_profile_hook = lambda: _the_hook
    sys.modules["antenv.axon_hooks"] = _mod

bass_utils.upload_artifacts = lambda tmpdir: str(tmpdir)

_MAXW = 1
if not getattr(bass.Bass, "_wait_split_installed", False):
    _orig_to_json_bytes = bass.Bass.to_json_bytes

    def _split_sync_waits(self, *a, **kw):
        bir = json.loads(_orig_to_json_bytes(self, *a, **kw))
        for fn in bir.get("functions", []):
            for blk in fn.get("blocks", []):
                new_insts = []
                for inst in blk.get("instructions", []):
                    si = inst.get("sync_info") or {}
                    waits = si.get("on_wait") or []
                    if len(waits) > _MAXW:
                        extra, keep = waits[:-_MAXW], waits[-_MAXW:]
                        for k in range(0, len(extra), _MAXW):
                            new_insts.append({
                                "debug": inst.get("debug", 0),
                                "engine": inst["engine"],
                                "ins": [], "outs": [],
                                "name": f"{inst['name']}_wsplit{k}",
                                "opcode": "NoOp",
                                "sync_info": {"on_update": [],
                                              "on_wait": extra[k:k + _MAXW]},
                            })
                        si["on_wait"] = keep
                    new_insts.append(inst)
                blk["instructions"] = new_insts
        return json.dumps(bir).encode()

    bass.Bass.to_json_bytes = _split_sync_waits
    bass.Bass._wait_split_installed = True


# ---------------------------------------------------------------------------
# Device kernel
# ---------------------------------------------------------------------------
def _build_nc():
    nc = bass.Bass("TRN2", target_bir_lowering=False, debug=False)
    featd = nc.declare_dram_parameter("featT", [NE, E, BS], FP8, isOutput=False)
    targd = nc.declare_dram_parameter("targT", [NE, E, BS], FP8, isOutput=False)
    iod = nc.declare_dram_parameter("ioT", [NE, E, 2, BS], FP8, isOutput=False)
    w1d = nc.declare_dram_parameter("w1", [E, NE, H], FP8, isOutput=False)
    w2d = nc.declare_dram_parameter("w2", [128, NE, 2, E], FP8, isOutput=False)
    # head1 = [b1(f32-as-bytes) | W1[e0]] unblocks the first mm1+ACT;
    # head2 = [negI | W2[e0]] unblocks the first flush (needed ~6us later).
    head1d = nc.declare_dram_parameter("head1", [128, 4, 128], FP8, isOutput=False)
    head2d = nc.declare_dram_parameter("head2", [128, 3, 128], FP8, isOutput=False)
    statsd = nc.declare_dram_parameter("stats", [128, 4 * NE, 6], F32, isOutput=True)

    GE = 8                     # experts per weight-DMA group
    NG = NE // GE

    with tile.TileContext(nc) as tc, contextlib.ExitStack() as ctx:
        wpool = ctx.enter_context(tc.tile_pool(name="weights", bufs=1,
                                                side="left"))
        # io (the DMA landing zone) lives at the far end of SBUF from the
        # tiles the engines stream (hact, weights): concurrent DMA landings
        # measurably slow ACT/PE ops touching nearby addresses.
        iopool = ctx.enter_context(tc.tile_pool(name="io", bufs=4,
                                                side="right"))
        hpool = ctx.enter_context(tc.tile_pool(name="hact", bufs=3,
                                               side="left"))
        php = ctx.enter_context(tc.tile_pool(name="ph", bufs=2, space="PSUM"))
        ppp = ctx.enter_context(tc.tile_pool(name="pp", bufs=2, space="PSUM"))

        head1_sb = wpool.tile([128, 4, 128], FP8)
        b1f = head1_sb[:, 0:2, :].bitcast(F32)       # [128, 2, 32] (p, c, n)
        w1e0 = head1_sb[:, 2:4, :]                   # [128, 2, 128] (p, c, m)
        head2_sb = wpool.tile([128, 3, 128], FP8)
        negi_sb = head2_sb[:, 0, :]
        w2e0 = head2_sb[:, 1:3, :]                   # [128, 2, 128] DR lhsT
        w1_sb = wpool.tile([E, NE, H], FP8)          # [128, 32, 256]
        w2_sb = wpool.tile([128, NE, 2, E], FP8)
        stats_sb = wpool.tile([128, 4 * NE, 6], F32)
        warm_sb = wpool.tile([128, 1], F32)

        # Ramp DMAs fan out across queues so the transfers overlap:
        #   scalar (HWDGE ring 2): head1 (critical), head2
        #   sync   (HWDGE ring 1): feat0 halves, targ0, feat1..3, steady state
        #   gpsimd:                memsets only (a SWDGE dma here inserts a
        #                          multi-us DRAIN that delays the PE warm-up)
        junk_sb = wpool.tile([128, 512], FP8)
        nc.gpsimd.memset(warm_sb[:], 0.0)
        nc.gpsimd.memset(junk_sb[:], 0.0)
        nc.scalar.dma_start(out=head1_sb[:], in_=head1d[:])
        # gelu table load + pipe warm during the DMA ramp
        nc.scalar.activation(warm_sb[:], warm_sb[:],
                             mybir.ActivationFunctionType.Gelu)
        # rest of the ramp weights ride the Act HWDGE ring, behind head1,
        # deadline-ordered: W1[e1] (needed ~15us) goes as a small transfer
        # ahead of the big group-0 blocks.
        nc.scalar.dma_start(out=w1_sb[:, 1:2, :], in_=w1d[:, 1:2, :])
        nc.scalar.dma_start(out=head2_sb[:], in_=head2d[:])
        nc.scalar.dma_start(out=w1_sb[:, 2:GE, :], in_=w1d[:, 2:GE, :])
        nc.scalar.dma_start(out=w2_sb[:, ts(0, GE), :, :], in_=w2d[:, ts(0, GE), :, :])

        def w1ap(n, c):
            return w1e0[:, c, :] if n == 0 else w1_sb[:, n, ts(c, 128)]

        def w2ap(n):
            return w2e0 if n == 0 else w2_sb[:, n, :, :]

        # flush tile t of expert nf: mm2 (DoubleRow) + negI + bn
        def flush_tile(nf, hact, targ_sb, t, dst):
            nc.tensor.matmul(dst, lhsT=w2ap(nf),
                             rhs=hact[t // 2][:, t % 2, :, :],
                             start=True, stop=False, perf_mode=DR)
            nc.tensor.matmul(dst, lhsT=negi_sb,
                             rhs=targ_sb[:, ts(t, BT)],
                             start=False, stop=True)
            nc.vector.bn_stats(out=stats_sb[:, 4 * nf + t, :], in_=dst)

        feat_tiles = {}

        def fetch_feat(n):
            if n < NE and n not in feat_tiles:
                if n == 0:
                    # two tiles so mm1(t0/t1) only waits the first half
                    fa = wpool.tile([E, 2 * BT], FP8, name="feat0a")
                    fb = wpool.tile([E, 2 * BT], FP8, name="feat0b")
                    nc.sync.dma_start(out=fa[:], in_=featd[0][:, 0:2 * BT])
                    nc.sync.dma_start(out=fb[:], in_=featd[0][:, 2 * BT:])
                    feat_tiles[0] = (fa, fb)
                else:
                    # feat+targ as ONE transfer: halves the landing events
                    # that each cost ~110ns on a concurrent ACT op
                    f = iopool.tile([E, 2, BS], FP8, tag="io", name="io_sb")
                    nc.sync.dma_start(out=f[:], in_=iod[n])
                    feat_tiles[n] = f

        fetch_feat(0)
        fetch_feat(1)
        targ0_sb = iopool.tile([E, BS], FP8, tag="targ")
        nc.sync.dma_start(out=targ0_sb[:], in_=targd[0])
        fetch_feat(2)
        fetch_feat(3)
        # PE warm-up: keep the tensor engine busy through the DMA ramp so the
        # first real matmuls run at full p-state instead of cold-start speed.
        warm_ps = php.tile([128, 2, BT], F32, name="warmps", tag="ph")
        for i in range(10):
            nc.tensor.matmul(warm_ps[:, i % 2, 0:256], lhsT=junk_sb[:, 0:128],
                             rhs=junk_sb[:, 0:256], start=True, stop=True)

        prev = None            # (n-1, hact, targ_sb)
        for n in range(NE):
            fetch_feat(n)
            feat_sb = feat_tiles.pop(n)

            def feat_ap(n_, t_):
                if n_ == 0:
                    return feat_sb[t_ // 2][:, ts(t_ % 2, BT)]
                return feat_sb[:, 0, ts(t_, BT)]

            fetch_feat(n + 1)
            fetch_feat(n + 2)
            if n >= 2:
                # depth-3 only once the ramp burst has drained: extra
                # in-flight transfers during experts 0-1 slow their mm1s
                fetch_feat(n + 3)
            targ_sb = targ0_sb if n == 0 else feat_sb[:, 1, :]
            if 4 <= n < 4 + 2 * (NG - 1):
                g, which = divmod(n - 4, 2)
                g += 1
                if which == 0:
                    nc.sync.dma_start(out=w1_sb[:, ts(g, GE), :],
                                      in_=w1d[:, ts(g, GE), :])
                else:
                    nc.sync.dma_start(out=w2_sb[:, ts(g, GE), :, :],
                                      in_=w2d[:, ts(g, GE), :, :])
            if n == 25:
                # first 3/4 of the stats is final; overlap the store
                nc.sync.dma_start(out=statsd[:, 0:96, :],
                                  in_=stats_sb[:, 0:96, :])

            h01 = hpool.tile([128, 2, 2, BT], FP8, tag="h01")
            h23 = hpool.tile([128, 2, 2, BT], FP8, tag="h23")
            hact = (h01, h23)
            p01 = p23 = None
            for g in range(4):
                c, pr = divmod(g, 2)
                ph_t = php.tile([128, 2, BT], F32, name=f"ph{g}", tag="ph")
                for i in range(2):
                    nc.tensor.matmul(ph_t[:, i, :], lhsT=w1ap(n, c),
                                     rhs=feat_ap(n, 2 * pr + i),
                                     start=True, stop=True)
                nc.scalar.activation(hact[pr][:, :, c, :],
                                     ph_t[:, :, :],
                                     mybir.ActivationFunctionType.Gelu,
                                     bias=b1f[:, c, n:n + 1], scale=ACT_SCALE)
                if prev is not None:
                    if g == 0:
                        p01 = ppp.tile([128, 2, BT], F32, name="p01", tag="pp")
                        flush_tile(prev[0], prev[1], prev[2], 0, p01[:, 0, :])
                    elif g == 1:
                        flush_tile(prev[0], prev[1], prev[2], 1, p01[:, 1, :])
                    elif g == 2:
                        p23 = ppp.tile([128, 2, BT], F32, name="p23", tag="pp")
                        flush_tile(prev[0], prev[1], prev[2], 2, p23[:, 0, :])
                    else:
                        flush_tile(prev[0], prev[1], prev[2], 3, p23[:, 1, :])
                        if n == NE - 1:
                            # tail compression: e31's t01 pair is complete
                            # after ACT(g2), so flush it here; only t2/t3
                            # remain after the final gelu op.
                            pf01 = ppp.tile([128, 2, BT], F32,
                                            name="pf01", tag="pp")
                            flush_tile(n, hact, targ_sb, 0, pf01[:, 0, :])
                            flush_tile(n, hact, targ_sb, 1, pf01[:, 1, :])
                            nc.sync.dma_start(out=statsd[:, 96:126, :],
                                              in_=stats_sb[:, 96:126, :])
            prev = (n, hact, targ_sb)
        pf23 = ppp.tile([128, 2, BT], F32, name="pf23", tag="pp")
        flush_tile(prev[0], prev[1], prev[2], 2, pf23[:, 0, :])
        flush_tile(prev[0], prev[1], prev[2], 3, pf23[:, 1, :])
        nc.sync.dma_start(out=statsd[:, 126:, :], in_=stats_sb[:, 126:, :])
    return nc


LAST_RESULTS = None


def kernel(features, target_features, W1, b1, W2, b2):
    global LAST_RESULTS
    f8 = ml_dtypes.float8_e4m3
    features = np.asarray(features)
    target_features = np.asarray(target_features)
    W1 = np.asarray(W1)
    b1 = np.asarray(b1)
    W2 = np.asarray(W2)
    b2 = np.asarray(b2)

    feat4 = (features * S_X).reshape(C, BS, NE, E).transpose(0, 2, 3, 1).astype(f8)
    targ4 = ((target_features - b2[None]) * S_W2).reshape(C, BS, NE, E) \
        .transpose(0, 2, 3, 1).astype(f8)
    w1h = (W1 * S_W1).transpose(1, 0, 2).astype(f8)                  # [E, NE, H]
    w2h = (W2 * S_W2).reshape(NE, 2, 128, E).transpose(2, 0, 1, 3).astype(f8)
    b1h = np.ascontiguousarray(
        b1.reshape(NE, 2, 128).transpose(2, 1, 0).astype(np.float32))  # [p, c, n]

    negi = (-np.eye(128)).astype(f8)
    head1 = np.ascontiguousarray(np.concatenate(
        [b1h.view(np.uint8).reshape(128, 256),
         np.ascontiguousarray(w1h[:, 0, :]).view(np.uint8)],
        axis=1)).view(f8).reshape(128, 4, 128)
    head2 = np.ascontiguousarray(np.concatenate(
        [negi.view(np.uint8),
         np.ascontiguousarray(w2h[:, 0, :, :]).reshape(128, 256).view(np.uint8)],
        axis=1)).view(f8).reshape(128, 3, 128)

    io4 = np.stack([feat4, targ4], axis=3)           # [C, NE, E, 2, BS]
    nc = _build_nc()
    in_maps = [
        {"featT": np.ascontiguousarray(feat4[c]),
         "targT": np.ascontiguousarray(targ4[c]),
         "ioT": np.ascontiguousarray(io4[c]),
         "w1": w1h, "w2": w2h, "head1": head1, "head2": head2}
        for c in range(C)
    ]
    res = run_bass_kernel_spmd(nc, in_maps, list(range(C)))
    LAST_RESULTS = res
    # stats[p, slot] = [n0, mean0, M2_0, n1, mean1, M2_1] over the two
    # 256-halves of 16*diff; sum(diff^2) = (M2 + n*mean^2) / 256.
    total = 0.0
    for r in res.results:
        st = r["stats"].astype(np.float64)
        total += (st[..., 2] + st[..., 0] * st[..., 1] ** 2
                  + st[..., 5] + st[..., 3] * st[..., 4] ** 2).sum()
    return np.array(total * TTR_SCALE / (B * NE * E), dtype=np.float32)
